# revision 2
# baseline (speedup 1.0000x reference)
"""Distributed 2-layer GCN on 8 Trainium2 NeuronCores (Bass/Tile).

Strategy (node partition over 8 cores, host-mediated halo exchange):
  Launch A: per-core T = x_shard @ W1            (dense matmul, fp16)
  host:     allgather T shards -> T_full, restage to every core
  Launch B: per-core aggregation for its dst nodes:
              dma_gather T_full[src] rows per edge (GPSIMD ucode gather),
              segment-sum via one-hot(norm) S-matrix matmuls into PSUM,
              h = relu(agg + b1), Z^T = W2^T @ h^T
  host:     allgather Z shards -> Z_full, restage
  Launch C: per-core dma_gather Z_full[src] rows, same S-matmul
              aggregation, out^T = agg + b2

dma_gather uses int16 indices, so the node table is addressed as a lo half
(rows < 32768) and a hi half; each block's edges are segregated lo-first.
Self-loop messages are fetched with one affine DMA per block (the per-core
node table restaged in dst-slot order) instead of gather indices.
All normalization (D^-1/2 A D^-1/2, with self-loops) is folded into the
per-edge S values. Host work is index bookkeeping, dtype casts and
concatenation only; all FLOPs and feature traffic run on the NeuronCores.
"""

import heapq
import os
import sys
import types

import numpy as np

import concourse.bass as bass
import concourse.bacc as bacc
import concourse.tile as tile
from concourse import mybir
from concourse.bass_utils import run_bass_kernel_spmd

NCORES = 8
N = 50000
FIN = 768
HID = 256
FOUT = 128
NLOC = N // NCORES            # 6250 nodes per core
NBLK = 49                     # dst blocks per core (49*128 = 6272 slots)
NLOC_PAD = NBLK * 128
P = 128
NLO = 32768                   # lo table rows (int16 index range)
GMAX = 1024                   # max indices per dma_gather instruction
NQ = 4                        # SWDGE queues (ucode max)
NOTRIM_BLOCKS = NBLK          # -1 (trim) gather indices abort DMA on current runtime;
                              # pad with row 0 everywhere instead

F16 = mybir.dt.float16
F32 = mybir.dt.float32
I16 = mybir.dt.int16

_KC = FIN // P  # 6


def _ensure_ntff_hook():
    """Provide antenv.axon_hooks if the image lacks it, so trace=True works."""
    try:
        import antenv.axon_hooks  # noqa: F401
        return
    except ImportError:
        pass
    import antenv
    mod = types.ModuleType("antenv.axon_hooks")
    mod._hook = None

    def set_axon_ntff_profile_hook(hook):
        mod._hook = hook

    def get_axon_ntff_profile_hook():
        return mod._hook

    mod.set_axon_ntff_profile_hook = set_axon_ntff_profile_hook
    mod.get_axon_ntff_profile_hook = get_axon_ntff_profile_hook
    sys.modules["antenv.axon_hooks"] = mod
    antenv.axon_hooks = mod
    try:
        from trn_agent_boot.trn_boot import _ntff_profile_via_ctypes
        hook = _ntff_profile_via_ctypes("/opt/axon/libaxon_pjrt.so")
        if hook is not None:
            mod._hook = hook
    except Exception:
        pass


def _wrap16(idx, ncols, pad):
    """dma_gather index staging: idx i -> [i % 16, i // 16], tiled to 128 rows."""
    a = np.full(ncols * 16, pad, np.int16)
    a[:len(idx)] = idx
    return np.tile(a.reshape(ncols, 16).T, (8, 1))    # [128, ncols]


def _preprocess(edge_index):
    """Partition edges by dst core; bin-pack dsts into 128-slot blocks with
    balanced edge counts; segregate lo/hi srcs; build staging arrays.
    Self-loops are NOT in the edge lists (handled by the affine self chunk)."""
    src = edge_index[0].astype(np.int64)
    dst = edge_index[1].astype(np.int64)
    loops = np.arange(N, dtype=np.int64)
    deg = np.bincount(np.concatenate([dst, loops]), minlength=N).astype(np.float64)
    dinv = 1.0 / np.sqrt(deg)                       # deg >= 1 (self-loops)
    norm_all = (dinv[src] * dinv[dst]).astype(np.float32)
    norm_self = (dinv * dinv).astype(np.float32)    # self-loop weights

    cores = []
    max_lo = 0
    max_hi = 0
    for c in range(NCORES):
        lo, hi = c * NLOC, (c + 1) * NLOC
        sel = (dst >= lo) & (dst < hi)
        s_c = src[sel]
        d_c = dst[sel] - lo
        n_c = norm_all[sel]

        degc = np.bincount(d_c, minlength=NLOC)
        # LPT bin-pack: heaviest dst first into the lightest block (<128 dsts)
        order = np.argsort(-degc, kind="stable")
        block_of = np.empty(NLOC, np.int64)
        slot_of = np.empty(NLOC, np.int64)
        heap = [(0, 0, b) for b in range(NBLK)]     # (edge count, slots used, b)
        heapq.heapify(heap)
        spill = []
        for dnode in order:
            while True:
                cnt, slots, b = heapq.heappop(heap)
                if slots < P:
                    break
                spill.append((cnt, slots, b))
            block_of[dnode] = b
            slot_of[dnode] = slots
            heapq.heappush(heap, (cnt + int(degc[dnode]), slots + 1, b))
            for it in spill:
                heapq.heappush(heap, it)
            spill.clear()

        eb = block_of[d_c]
        es = slot_of[d_c]
        is_hi = (s_c >= NLO).astype(np.int64)
        o = np.lexsort((es, is_hi, eb))             # block, then lo|hi, then slot
        s_c, n_c, eb, es, is_hi = s_c[o], n_c[o], eb[o], es[o], is_hi[o]
        cnt_lo = np.bincount(eb[is_hi == 0], minlength=NBLK)
        cnt_hi = np.bincount(eb[is_hi == 1], minlength=NBLK)
        max_lo = max(max_lo, int(cnt_lo.max()))
        max_hi = max(max_hi, int(cnt_hi.max()))
        cores.append((s_c, n_c, eb, es, is_hi, cnt_lo, cnt_hi, block_of, slot_of))

    clo = (max_lo + P - 1) // P                     # lo chunks per block
    chi = (max_hi + P - 1) // P                     # hi chunks per block
    cpbt = 1 + clo + chi                            # chunk 0 = affine self chunk

    def _splits(nch):
        out = []
        j = 0
        while j < nch:
            n = min(GMAX // P, nch - j)
            out.append((j, n))
            j += n
        return out
    lo_splits = _splits(clo)
    hi_splits = _splits(chi)

    out = []
    for c, (s_c, n_c, eb, es, is_hi, cnt_lo, cnt_hi, block_of, slot_of) in \
            enumerate(cores):
        cap_lo, cap_hi = clo * P, chi * P
        cap = cpbt * P
        cum_lo = np.concatenate([[0], np.cumsum(cnt_lo)])
        cum_hi = np.concatenate([[0], np.cumsum(cnt_hi)])
        nedge = len(eb)
        pos_in_half = np.empty(nedge, np.int64)
        m_lo = is_hi == 0
        idx_lo = np.nonzero(m_lo)[0]
        idx_hi = np.nonzero(~m_lo)[0]
        pos_in_half[idx_lo] = np.arange(len(idx_lo)) - cum_lo[eb[idx_lo]]
        pos_in_half[idx_hi] = np.arange(len(idx_hi)) - cum_hi[eb[idx_hi]]
        pos = P + np.where(m_lo, pos_in_half, cap_lo + pos_in_half)  # +P: self chunk
        flat = eb * cap + pos

        srcg = np.zeros((NBLK, cap), np.int64)      # absolute src (pads -> row 0)
        srcg[:, P + cap_lo:] = NLO
        dstslot = np.full((NBLK, cap), -1.0, np.float16)
        normv = np.zeros((NBLK, cap), np.float16)
        srcg.reshape(-1)[flat] = s_c
        dstslot.reshape(-1)[flat] = es.astype(np.float16)
        normv.reshape(-1)[flat] = n_c.astype(np.float16)

        # self chunk (positions 0..127 of each block): slot p <- node at (b, p)
        node_at = np.full((NBLK, P), -1, np.int64)
        node_at[block_of, slot_of] = np.arange(NLOC)
        used = node_at >= 0
        dstslot[:, :P] = np.where(used, np.arange(P)[None, :], -1).astype(np.float16)
        normv[:, :P] = np.where(
            used, norm_self[c * NLOC + np.where(used, node_at, 0)], 0.0
        ).astype(np.float16)
        srcg[:, :P] = np.where(used, c * NLOC + np.where(used, node_at, 0), 0)

        # int16 wrapped index staging per (block, half, split)
        idxlo = np.zeros((NBLK, P, clo * 8), np.int16)
        idxhi = np.zeros((NBLK, P, chi * 8), np.int16)
        for b in range(NBLK):
            trim = b >= NOTRIM_BLOCKS
            for (j0, nch) in lo_splits:
                nreal = min(max(int(cnt_lo[b]) - j0 * P, 0), nch * P)
                seg = srcg[b, P + j0 * P: P + j0 * P + nreal].astype(np.int16)
                idxlo[b, :, j0 * 8:(j0 + nch) * 8] = _wrap16(
                    seg, nch * 8, -1 if trim else 0)
            for (j0, nch) in hi_splits:
                nreal = min(max(int(cnt_hi[b]) - j0 * P, 0), nch * P)
                seg = (srcg[b, P + cap_lo + j0 * P: P + cap_lo + j0 * P + nreal]
                       - NLO).astype(np.int16)
                idxhi[b, :, j0 * 8:(j0 + nch) * 8] = _wrap16(
                    seg, nch * 8, -1 if trim else 0)

        dstslot = np.ascontiguousarray(
            dstslot.reshape(NBLK, cpbt, P).transpose(0, 2, 1))
        normv = np.ascontiguousarray(
            normv.reshape(NBLK, cpbt, P).transpose(0, 2, 1))
        srcg = np.ascontiguousarray(
            srcg.reshape(NBLK, cpbt, P).transpose(0, 2, 1)).astype(np.int32)
        perm = (block_of * P + slot_of).astype(np.int64)
        out.append({"idxlo": idxlo, "idxhi": idxhi, "dstslot": dstslot,
                    "normv": normv, "perm": perm, "srcg": srcg})
    return out, clo, chi, lo_splits, hi_splits


def _build_a():
    nc = bacc.Bacc("TRN2", target_bir_lowering=False, debug=False, num_devices=NCORES)
    # host-swizzled so each block loads as one contiguous-per-partition DMA:
    # xtb[b, p, k*128+n] = x[b*128+n, k*128+p]
    t_xt = nc.dram_tensor("xtb", [NBLK, P, FIN], F16, kind="ExternalInput")
    t_w1 = nc.dram_tensor("w1", [FIN, HID], F16, kind="ExternalInput")
    t_out = nc.dram_tensor("t_out", [NLOC_PAD, HID], F16, kind="ExternalOutput")
    with tile.TileContext(nc) as tc:
        with (
            tc.tile_pool(name="const", bufs=1) as cs,
            tc.tile_pool(name="sb", bufs=4) as sb,
            tc.tile_pool(name="ps", bufs=2, space="PSUM") as ps,
        ):
            w1t = cs.tile([P, _KC * HID], F16)
            for k in range(_KC):
                nc.sync.dma_start(w1t[:, k * HID:(k + 1) * HID],
                                  t_w1[k * P:(k + 1) * P, :])
            for b in range(NBLK):
                xts = sb.tile([P, FIN], F16, tag="xt")
                nc.sync.dma_start(xts[:], t_xt[b])
                pt = ps.tile([P, HID], F32, tag="pt")
                for k in range(_KC):
                    nc.tensor.matmul(pt[:], lhsT=xts[:, k * P:(k + 1) * P],
                                     rhs=w1t[:, k * HID:(k + 1) * HID],
                                     start=(k == 0), stop=(k == _KC - 1))
                ts = sb.tile([P, HID], F16, tag="ts")
                nc.vector.tensor_copy(ts[:], pt[:])
                nc.sync.dma_start(t_out[b * P:(b + 1) * P, :], ts[:])
    nc.compile()
    return nc


def _build_agg(cfg):
    """Aggregation launch: B (elem=HID, relu+b1, then @W2 -> Z^T) or
    C (elem=FOUT, +b2 -> out^T)."""
    is_b = cfg["is_b"]
    clo, chi, lo_splits, hi_splits = (cfg["clo"], cfg["chi"],
                                      cfg["lo_splits"], cfg["hi_splits"])
    cpbt = 1 + clo + chi
    elem = HID if is_b else FOUT
    nc = bacc.Bacc("TRN2", target_bir_lowering=False, debug=False,
                   num_devices=NCORES, num_swdge_queues=NQ)
    t_tf = nc.dram_tensor("tfull", [N, elem], F16, kind="ExternalInput")
    t_sf = nc.dram_tensor("tself", [NLOC_PAD, elem], F16, kind="ExternalInput")
    t_il = nc.dram_tensor("idxlo", [NBLK, P, clo * 8], I16, kind="ExternalInput")
    t_ih = nc.dram_tensor("idxhi", [NBLK, P, chi * 8], I16, kind="ExternalInput")
    t_ds = nc.dram_tensor("dstslot", [NBLK, P, cpbt], F16, kind="ExternalInput")
    t_nm = nc.dram_tensor("normv", [NBLK, P, cpbt], F16, kind="ExternalInput")
    t_ci = nc.dram_tensor("colidx", [P, cpbt * P], F16, kind="ExternalInput")
    if is_b:
        t_w2 = nc.dram_tensor("w2", [HID, FOUT], F16, kind="ExternalInput")
        t_b1 = nc.dram_tensor("b1c", [P, 2], F32, kind="ExternalInput")
        t_o = nc.dram_tensor("zt_out", [FOUT, NLOC_PAD], F16, kind="ExternalOutput")
    else:
        t_b2 = nc.dram_tensor("b2c", [P, 1], F32, kind="ExternalInput")
        t_o = nc.dram_tensor("ot_out", [FOUT, NLOC_PAD], F32, kind="ExternalOutput")

    tf_lo = t_tf[0:NLO, :]
    tf_hi = t_tf[NLO:N, :]
    qn = [0]

    def _next_q():
        q = qn[0] % NQ
        qn[0] += 1
        return q

    with tile.TileContext(nc) as tc:
        with (
            tc.tile_pool(name="const", bufs=1) as cs,
            tc.tile_pool(name="sb", bufs=3) as sb,
            tc.tile_pool(name="ps", bufs=2, space="PSUM") as ps,
        ):
            colidx = cs.tile([P, cpbt * P], F16)
            nc.sync.dma_start(colidx[:], t_ci[:])
            if is_b:
                w2t = cs.tile([P, 2 * FOUT], F16)
                for k in range(2):
                    nc.sync.dma_start(w2t[:, k * FOUT:(k + 1) * FOUT],
                                      t_w2[k * P:(k + 1) * P, :])
                b1t = cs.tile([P, 2], F32)
                nc.sync.dma_start(b1t[:], t_b1[:])
            else:
                b2t = cs.tile([P, 1], F32)
                nc.sync.dma_start(b2t[:], t_b2[:])

            for b in range(NBLK):
                il = sb.tile([P, clo * 8], I16, tag="il")
                nc.sync.dma_start(il[:], t_il[b])
                ih = sb.tile([P, chi * 8], I16, tag="ih")
                nc.sync.dma_start(ih[:], t_ih[b])
                dst = sb.tile([P, cpbt], F16, tag="ds")
                nc.sync.dma_start(dst[:], t_ds[b])
                nrm = sb.tile([P, cpbt], F16, tag="nm")
                nc.sync.dma_start(nrm[:], t_nm[b])

                g = sb.tile([P, cpbt * elem], F16, tag="g")
                g3 = g[:].rearrange("p (c e) -> p c e", e=elem)
                # chunk 0: self loops, affine fetch from slot-ordered table
                nc.sync.dma_start(g[:, 0:elem], t_sf[b * P:(b + 1) * P, :])
                for (j0, nch) in lo_splits:
                    nc.gpsimd.dma_gather(
                        out_ap=g3[:, 1 + j0:1 + j0 + nch, :],
                        in_ap=tf_lo,
                        idxs_ap=il[:, j0 * 8:(j0 + nch) * 8],
                        num_idxs=nch * P,
                        num_idxs_reg=nch * P,
                        elem_size=elem,
                        queue_num=_next_q(),
                    )
                for (j0, nch) in hi_splits:
                    nc.gpsimd.dma_gather(
                        out_ap=g3[:, 1 + clo + j0:1 + clo + j0 + nch, :],
                        in_ap=tf_hi,
                        idxs_ap=ih[:, j0 * 8:(j0 + nch) * 8],
                        num_idxs=nch * P,
                        num_idxs_reg=nch * P,
                        elem_size=elem,
                        queue_num=_next_q(),
                    )

                seq = sb.tile([P, cpbt * P], F16, tag="seq")
                nc.vector.tensor_tensor(
                    out=seq[:].rearrange("p (c s) -> p c s", c=cpbt),
                    in0=dst[:].to_broadcast([P, cpbt, P]),
                    in1=colidx[:].rearrange("p (c s) -> p c s", c=cpbt),
                    op=mybir.AluOpType.is_equal)
                s = sb.tile([P, cpbt * P], F16, tag="s")
                nc.vector.tensor_tensor(
                    out=s[:].rearrange("p (c s) -> p c s", c=cpbt),
                    in0=seq[:].rearrange("p (c s) -> p c s", c=cpbt),
                    in1=nrm[:].to_broadcast([P, cpbt, P]),
                    op=mybir.AluOpType.mult)

                if is_b:
                    h0p = ps.tile([P, P], F32, tag="h0p")
                    h1p = ps.tile([P, P], F32, tag="h1p")
                    for j in range(cpbt):
                        nc.tensor.matmul(h0p[:], lhsT=g[:, j * elem:j * elem + P],
                                         rhs=s[:, j * P:(j + 1) * P],
                                         start=(j == 0), stop=(j == cpbt - 1))
                        nc.tensor.matmul(h1p[:], lhsT=g[:, j * elem + P:(j + 1) * elem],
                                         rhs=s[:, j * P:(j + 1) * P],
                                         start=(j == 0), stop=(j == cpbt - 1))
                    h0 = sb.tile([P, P], F16, tag="h0")
                    nc.scalar.activation(out=h0[:], in_=h0p[:],
                                         func=mybir.ActivationFunctionType.Relu,
                                         bias=b1t[:, 0:1], scale=1.0)
                    h1 = sb.tile([P, P], F16, tag="h1")
                    nc.scalar.activation(out=h1[:], in_=h1p[:],
                                         func=mybir.ActivationFunctionType.Relu,
                                         bias=b1t[:, 1:2], scale=1.0)
                    zp = ps.tile([P, P], F32, tag="zp")
                    nc.tensor.matmul(zp[:], lhsT=w2t[:, 0:FOUT], rhs=h0[:],
                                     start=True, stop=False)
                    nc.tensor.matmul(zp[:], lhsT=w2t[:, FOUT:2 * FOUT], rhs=h1[:],
                                     start=False, stop=True)
                    z = sb.tile([P, P], F16, tag="z")
                    nc.vector.tensor_copy(z[:], zp[:])
                    nc.sync.dma_start(t_o[:, b * P:(b + 1) * P], z[:])
                else:
                    op_ = ps.tile([P, P], F32, tag="op")
                    for j in range(cpbt):
                        nc.tensor.matmul(op_[:], lhsT=g[:, j * elem:(j + 1) * elem],
                                         rhs=s[:, j * P:(j + 1) * P],
                                         start=(j == 0), stop=(j == cpbt - 1))
                    ot = sb.tile([P, P], F32, tag="ot")
                    nc.scalar.activation(out=ot[:], in_=op_[:],
                                         func=mybir.ActivationFunctionType.Identity,
                                         bias=b2t[:, 0:1], scale=1.0)
                    nc.sync.dma_start(t_o[:, b * P:(b + 1) * P], ot[:])
    nc.compile()
    return nc


_KERNEL_CACHE = {}


def _get_kernels(clo, chi, lo_splits, hi_splits):
    key = (clo, chi)
    if key not in _KERNEL_CACHE:
        cfg = dict(clo=clo, chi=chi, lo_splits=lo_splits, hi_splits=hi_splits)
        _KERNEL_CACHE[key] = (
            _build_a(),
            _build_agg({**cfg, "is_b": True}),
            _build_agg({**cfg, "is_b": False}),
        )
    return _KERNEL_CACHE[key]


def kernel(x, edge_index, W1, b1, W2, b2):
    trace = bool(int(os.environ.get("GCN_TRACE", "0")))
    if trace:
        _ensure_ntff_hook()
    exec_ns = []

    def _run(nc, in_maps):
        res = run_bass_kernel_spmd(nc, in_maps, core_ids=list(range(NCORES)),
                                   trace=trace)
        if trace:
            exec_ns.append(res.exec_time_ns)
        return res.results

    x = np.asarray(x)
    edge_index = np.asarray(edge_index)
    W1 = np.asarray(W1, np.float32)
    b1 = np.asarray(b1, np.float32)
    W2 = np.asarray(W2, np.float32)
    b2 = np.asarray(b2, np.float32)

    pre, clo, chi, lo_splits, hi_splits = _preprocess(edge_index)
    cpbt = 1 + clo + chi
    nc_a, nc_b, nc_c = _get_kernels(clo, chi, lo_splits, hi_splits)

    # ---- launch A: T = x @ W1 (per-core node shard) ----
    w1_f16 = W1.astype(np.float16)
    in_a = []
    for c in range(NCORES):
        xs = np.zeros((NLOC_PAD, FIN), np.float16)
        xs[:NLOC] = x[c * NLOC:(c + 1) * NLOC].astype(np.float16)
        xtb = np.ascontiguousarray(
            xs.reshape(NBLK, P, _KC, P).transpose(0, 3, 2, 1).reshape(NBLK, P, FIN))
        in_a.append({"xtb": xtb, "w1": w1_f16})
    res_a = _run(nc_a, in_a)
    tfull = np.concatenate([res_a[c]["t_out"][:NLOC] for c in range(NCORES)], axis=0)
    tfull = np.ascontiguousarray(tfull)            # [N, HID] f16

    colidx = np.tile(np.arange(P, dtype=np.float16)[None, :], (P, cpbt))
    colidx = np.ascontiguousarray(colidx.reshape(P, cpbt * P))

    # ---- launch B: h = relu(agg(T) + b1); Z^T = W2^T h^T ----
    w2_f16 = W2.astype(np.float16)
    b1c = np.stack([b1[:P], b1[P:]], axis=1).astype(np.float32)
    in_b = []
    for c in range(NCORES):
        tself = np.zeros((NLOC_PAD, HID), np.float16)
        tself[pre[c]["perm"]] = tfull[c * NLOC:(c + 1) * NLOC]
        in_b.append({
            "tfull": tfull, "tself": tself,
            "idxlo": pre[c]["idxlo"], "idxhi": pre[c]["idxhi"],
            "dstslot": pre[c]["dstslot"], "normv": pre[c]["normv"],
            "colidx": colidx, "w2": w2_f16, "b1c": b1c,
        })
    res_b = _run(nc_b, in_b)
    zts = [res_b[c]["zt_out"] for c in range(NCORES)]
    zfull = np.concatenate(
        [zts[c].T[pre[c]["perm"]] for c in range(NCORES)], axis=0)
    zfull = np.ascontiguousarray(zfull)            # [N, FOUT] f16

    # ---- launch C: out = agg(Z) + b2 ----
    b2c = b2[:, None].astype(np.float32)
    in_c = [{
        "tfull": zfull, "tself": np.ascontiguousarray(zts[c].T),
        "idxlo": pre[c]["idxlo"], "idxhi": pre[c]["idxhi"],
        "dstslot": pre[c]["dstslot"], "normv": pre[c]["normv"],
        "colidx": colidx, "b2c": b2c,
    } for c in range(NCORES)]
    res_c = _run(nc_c, in_c)
    out = np.concatenate(
        [res_c[c]["ot_out"].T[pre[c]["perm"]] for c in range(NCORES)], axis=0)

    if trace:
        ns = [int(t) if t else 0 for t in exec_ns]
        print(f"GCN launch exec times (ns): {ns}  total: {sum(ns)}")
        kernel.last_exec_ns = ns
    return np.ascontiguousarray(out.astype(np.float32))



# revision 12
# speedup vs baseline: 1.0288x; 1.0288x over previous
"""Distributed 2-layer GCN on 8 Trainium2 NeuronCores (Bass/Tile).

Strategy (node partition over 8 cores, host-mediated halo exchange):
  Launch A: per-core T = x_shard @ W1            (dense matmul, fp16)
  host:     allgather T shards -> T_full, restage to every core
  Launch B: per-core aggregation for its dst nodes:
              dma_gather T_full[src] rows per edge (GPSIMD ucode gather),
              segment-sum via one-hot(norm) S-matrix matmuls into PSUM,
              h = relu(agg + b1), Z^T = W2^T @ h^T
  host:     allgather Z shards -> Z_full, restage
  Launch C: per-core dma_gather Z_full[src] rows, same S-matmul
              aggregation, out^T = agg + b2

dma_gather uses int16 indices, so the node table is addressed as a lo half
(rows < 32768) and a hi half; each block's edges are segregated lo-first.
Self-loop messages are fetched with one affine DMA per block (the per-core
node table restaged in dst-slot order) instead of gather indices.
All normalization (D^-1/2 A D^-1/2, with self-loops) is folded into the
per-edge S values. Host work is index bookkeeping, dtype casts and
concatenation only; all FLOPs and feature traffic run on the NeuronCores.
"""

import os
import sys
import types

import numpy as np

import concourse.bass as bass
import concourse.bacc as bacc
import concourse.tile as tile
from concourse import mybir
from concourse.bass_utils import run_bass_kernel_spmd

NCORES = 8
N = 50000
FIN = 768
HID = 256
FOUT = 128
NLOC = N // NCORES            # 6250 nodes per core
NBLK = 49                     # dst blocks per core (49*128 = 6272 slots)
NLOC_PAD = NBLK * 128
P = 128
NLO = 32768                   # lo table rows (int16 index range)
GMAX = 1024                   # max indices per dma_gather instruction
NQ = 4                        # SWDGE queues (ucode max)
NOTRIM_BLOCKS = NBLK          # -1 (trim) gather indices abort DMA on current runtime;
                              # pad with row 0 everywhere instead

F16 = mybir.dt.float16
F32 = mybir.dt.float32
F8 = mybir.dt.float8e4
I16 = mybir.dt.int16

CAP_LO = 11 * P               # 2D bin-pack targets: lo chunks per block
CAP_HI = 6 * P                # hi chunks per block (11+6+self = 18 = cpbt)

_KC = FIN // P  # 6


def _ensure_ntff_hook():
    """Provide antenv.axon_hooks if the image lacks it, so trace=True works."""
    try:
        import antenv.axon_hooks  # noqa: F401
        return
    except ImportError:
        pass
    import antenv
    mod = types.ModuleType("antenv.axon_hooks")
    mod._hook = None

    def set_axon_ntff_profile_hook(hook):
        mod._hook = hook

    def get_axon_ntff_profile_hook():
        return mod._hook

    mod.set_axon_ntff_profile_hook = set_axon_ntff_profile_hook
    mod.get_axon_ntff_profile_hook = get_axon_ntff_profile_hook
    sys.modules["antenv.axon_hooks"] = mod
    antenv.axon_hooks = mod
    try:
        from trn_agent_boot.trn_boot import _ntff_profile_via_ctypes
        hook = _ntff_profile_via_ctypes("/opt/axon/libaxon_pjrt.so")
        if hook is not None:
            mod._hook = hook
    except Exception:
        pass


def _wrap16(idx, ncols, pad):
    """dma_gather index staging: idx i -> [i % 16, i // 16], tiled to 128 rows."""
    a = np.full(ncols * 16, pad, np.int16)
    a[:len(idx)] = idx
    return np.tile(a.reshape(ncols, 16).T, (8, 1))    # [128, ncols]


def _preprocess(edge_index):
    """Partition edges by dst core; bin-pack dsts into 128-slot blocks with
    balanced edge counts; segregate lo/hi srcs; build staging arrays.
    Self-loops are NOT in the edge lists (handled by the affine self chunk)."""
    src = edge_index[0].astype(np.int64)
    dst = edge_index[1].astype(np.int64)
    loops = np.arange(N, dtype=np.int64)
    deg = np.bincount(np.concatenate([dst, loops]), minlength=N).astype(np.float64)
    dinv = 1.0 / np.sqrt(deg)                       # deg >= 1 (self-loops)
    norm_all = (dinv[src] * dinv[dst]).astype(np.float32)
    norm_self = (dinv * dinv).astype(np.float32)    # self-loop weights

    cores = []
    max_lo = 0
    max_hi = 0
    for c in range(NCORES):
        lo, hi = c * NLOC, (c + 1) * NLOC
        sel = (dst >= lo) & (dst < hi)
        s_c = src[sel]
        d_c = dst[sel] - lo
        n_c = norm_all[sel]

        is_hi_d = s_c >= NLO
        dlo = np.bincount(d_c[~is_hi_d], minlength=NLOC)
        dhi = np.bincount(d_c[is_hi_d], minlength=NLOC)
        # 2D bin-pack: heaviest dst first into the lightest feasible block,
        # keeping every block under CAP_LO lo-edges AND CAP_HI hi-edges so
        # clo+chi stays minimal (ceil waste on both halves is shared).
        order = np.argsort(-(dlo + dhi), kind="stable")
        block_of = np.empty(NLOC, np.int64)
        slot_of = np.empty(NLOC, np.int64)
        blo = np.zeros(NBLK, np.int64)
        bhi = np.zeros(NBLK, np.int64)
        bsl = np.zeros(NBLK, np.int64)
        btot = np.zeros(NBLK, np.int64)
        for dnode in order:
            l, h = int(dlo[dnode]), int(dhi[dnode])
            open_ = bsl < P
            feas = open_ & (blo + l <= CAP_LO) & (bhi + h <= CAP_HI)
            if feas.any():
                b = int(np.where(feas, btot, np.iinfo(np.int64).max).argmin())
            else:   # overflow fallback: min cap excess, then lightest
                over = (np.maximum(blo + l - CAP_LO, 0)
                        + np.maximum(bhi + h - CAP_HI, 0))
                b = int(np.where(open_, over * (10 ** 7) + btot,
                                 np.iinfo(np.int64).max).argmin())
            block_of[dnode] = b
            slot_of[dnode] = bsl[b]
            blo[b] += l
            bhi[b] += h
            btot[b] += l + h
            bsl[b] += 1

        eb = block_of[d_c]
        es = slot_of[d_c]
        is_hi = (s_c >= NLO).astype(np.int64)
        o = np.lexsort((es, is_hi, eb))             # block, then lo|hi, then slot
        s_c, n_c, eb, es, is_hi = s_c[o], n_c[o], eb[o], es[o], is_hi[o]
        cnt_lo = np.bincount(eb[is_hi == 0], minlength=NBLK)
        cnt_hi = np.bincount(eb[is_hi == 1], minlength=NBLK)
        max_lo = max(max_lo, int(cnt_lo.max()))
        max_hi = max(max_hi, int(cnt_hi.max()))
        cores.append((s_c, n_c, eb, es, is_hi, cnt_lo, cnt_hi, block_of, slot_of))

    clo = (max_lo + P - 1) // P                     # lo chunks per block
    chi = (max_hi + P - 1) // P                     # hi chunks per block
    cpbt = 1 + clo + chi                            # chunk 0 = affine self chunk

    def _splits(nch):
        out = []
        j = 0
        while j < nch:
            n = min(GMAX // P, nch - j)
            out.append((j, n))
            j += n
        return out
    lo_splits = _splits(clo)
    hi_splits = _splits(chi)

    out = []
    for c, (s_c, n_c, eb, es, is_hi, cnt_lo, cnt_hi, block_of, slot_of) in \
            enumerate(cores):
        cap_lo, cap_hi = clo * P, chi * P
        cap = cpbt * P
        cum_lo = np.concatenate([[0], np.cumsum(cnt_lo)])
        cum_hi = np.concatenate([[0], np.cumsum(cnt_hi)])
        nedge = len(eb)
        pos_in_half = np.empty(nedge, np.int64)
        m_lo = is_hi == 0
        idx_lo = np.nonzero(m_lo)[0]
        idx_hi = np.nonzero(~m_lo)[0]
        pos_in_half[idx_lo] = np.arange(len(idx_lo)) - cum_lo[eb[idx_lo]]
        pos_in_half[idx_hi] = np.arange(len(idx_hi)) - cum_hi[eb[idx_hi]]
        pos = P + np.where(m_lo, pos_in_half, cap_lo + pos_in_half)  # +P: self chunk
        flat = eb * cap + pos

        srcg = np.zeros((NBLK, cap), np.int64)      # absolute src (pads -> row 0)
        srcg[:, P + cap_lo:] = NLO
        dstslot = np.full((NBLK, cap), -1.0, np.float16)
        normv = np.zeros((NBLK, cap), np.float16)
        srcg.reshape(-1)[flat] = s_c
        dstslot.reshape(-1)[flat] = es.astype(np.float16)
        normv.reshape(-1)[flat] = n_c.astype(np.float16)

        # self chunk (positions 0..127 of each block): slot p <- node at (b, p)
        node_at = np.full((NBLK, P), -1, np.int64)
        node_at[block_of, slot_of] = np.arange(NLOC)
        used = node_at >= 0
        dstslot[:, :P] = np.where(used, np.arange(P)[None, :], -1).astype(np.float16)
        normv[:, :P] = np.where(
            used, norm_self[c * NLOC + np.where(used, node_at, 0)], 0.0
        ).astype(np.float16)
        srcg[:, :P] = np.where(used, c * NLOC + np.where(used, node_at, 0), 0)

        # int16 wrapped index staging per (block, half, split)
        idxlo = np.zeros((NBLK, P, clo * 8), np.int16)
        idxhi = np.zeros((NBLK, P, chi * 8), np.int16)
        for b in range(NBLK):
            trim = b >= NOTRIM_BLOCKS
            for (j0, nch) in lo_splits:
                nreal = min(max(int(cnt_lo[b]) - j0 * P, 0), nch * P)
                seg = srcg[b, P + j0 * P: P + j0 * P + nreal].astype(np.int16)
                idxlo[b, :, j0 * 8:(j0 + nch) * 8] = _wrap16(
                    seg, nch * 8, -1 if trim else 0)
            for (j0, nch) in hi_splits:
                nreal = min(max(int(cnt_hi[b]) - j0 * P, 0), nch * P)
                seg = (srcg[b, P + cap_lo + j0 * P: P + cap_lo + j0 * P + nreal]
                       - NLO).astype(np.int16)
                idxhi[b, :, j0 * 8:(j0 + nch) * 8] = _wrap16(
                    seg, nch * 8, -1 if trim else 0)

        dstslot = np.ascontiguousarray(
            dstslot.reshape(NBLK, cpbt, P).transpose(0, 2, 1))
        normv = np.ascontiguousarray(
            normv.reshape(NBLK, cpbt, P).transpose(0, 2, 1))
        srcg = np.ascontiguousarray(
            srcg.reshape(NBLK, cpbt, P).transpose(0, 2, 1)).astype(np.int32)
        perm = (block_of * P + slot_of).astype(np.int64)
        out.append({"idxlo": idxlo, "idxhi": idxhi, "dstslot": dstslot,
                    "normv": normv, "perm": perm, "srcg": srcg})
    return out, clo, chi, lo_splits, hi_splits


def _build_a():
    nc = bacc.Bacc("TRN2", target_bir_lowering=False, debug=False, num_devices=NCORES)
    # host-swizzled so each block loads as one contiguous-per-partition DMA:
    # xtb[b, p, k*128+n] = x[b*128+n, k*128+p]
    t_xt = nc.dram_tensor("xtb", [NBLK, P, FIN], F16, kind="ExternalInput")
    t_w1 = nc.dram_tensor("w1", [FIN, HID], F16, kind="ExternalInput")
    t_out = nc.dram_tensor("t_out", [NLOC_PAD, HID], F8, kind="ExternalOutput")
    with tile.TileContext(nc) as tc:
        with (
            tc.tile_pool(name="const", bufs=1) as cs,
            tc.tile_pool(name="sb", bufs=4) as sb,
            tc.tile_pool(name="ps", bufs=2, space="PSUM") as ps,
        ):
            w1t = cs.tile([P, _KC * HID], F16)
            for k in range(_KC):
                nc.sync.dma_start(w1t[:, k * HID:(k + 1) * HID],
                                  t_w1[k * P:(k + 1) * P, :])
            for b in range(NBLK):
                xts = sb.tile([P, FIN], F16, tag="xt")
                nc.sync.dma_start(xts[:], t_xt[b])
                pt = ps.tile([P, HID], F32, tag="pt")
                for k in range(_KC):
                    nc.tensor.matmul(pt[:], lhsT=xts[:, k * P:(k + 1) * P],
                                     rhs=w1t[:, k * HID:(k + 1) * HID],
                                     start=(k == 0), stop=(k == _KC - 1))
                ts = sb.tile([P, HID], F8, tag="ts")
                nc.vector.tensor_copy(ts[:], pt[:])
                nc.sync.dma_start(t_out[b * P:(b + 1) * P, :], ts[:])
    nc.compile()
    return nc


def _build_agg(cfg):
    """Aggregation launch: B (elem=HID, relu+b1, then @W2 -> Z^T) or
    C (elem=FOUT, +b2 -> out^T)."""
    is_b = cfg["is_b"]
    clo, chi, lo_splits, hi_splits = (cfg["clo"], cfg["chi"],
                                      cfg["lo_splits"], cfg["hi_splits"])
    cpbt = 1 + clo + chi
    elem = HID if is_b else FOUT
    gd = F8 if is_b else F16      # layer-1 payload gathered in fp8 (256B rows)
    nc = bacc.Bacc("TRN2", target_bir_lowering=False, debug=False,
                   num_devices=NCORES, num_swdge_queues=NQ)
    t_tf = nc.dram_tensor("tfull", [N, elem], gd, kind="ExternalInput")
    t_sf = nc.dram_tensor("tself", [NLOC_PAD, elem], gd, kind="ExternalInput")
    t_il = nc.dram_tensor("idxlo", [NBLK, P, clo * 8], I16, kind="ExternalInput")
    t_ih = nc.dram_tensor("idxhi", [NBLK, P, chi * 8], I16, kind="ExternalInput")
    t_ds = nc.dram_tensor("dstslot", [NBLK, P, cpbt], F16, kind="ExternalInput")
    t_nm = nc.dram_tensor("normv", [NBLK, P, cpbt], F16, kind="ExternalInput")
    t_ci = nc.dram_tensor("colidx", [P, cpbt * P], F16, kind="ExternalInput")
    if is_b:
        t_w2 = nc.dram_tensor("w2", [HID, FOUT], F16, kind="ExternalInput")
        t_b1 = nc.dram_tensor("b1c", [P, 2], F32, kind="ExternalInput")
        t_o = nc.dram_tensor("zt_out", [FOUT, NLOC_PAD], F16, kind="ExternalOutput")
    else:
        t_b2 = nc.dram_tensor("b2c", [P, 1], F32, kind="ExternalInput")
        t_o = nc.dram_tensor("ot_out", [FOUT, NLOC_PAD], F32, kind="ExternalOutput")

    tf_lo = t_tf[0:NLO, :]
    tf_hi = t_tf[NLO:N, :]
    qn = [0]

    def _next_q():
        q = qn[0] % NQ
        qn[0] += 1
        return q

    with tile.TileContext(nc) as tc:
        with (
            tc.tile_pool(name="const", bufs=1) as cs,
            tc.tile_pool(name="sb", bufs=5) as sb,
            tc.tile_pool(name="ps", bufs=2, space="PSUM") as ps,
        ):
            colidx = cs.tile([P, cpbt * P], F16)
            nc.sync.dma_start(colidx[:], t_ci[:])
            if is_b:
                w2t = cs.tile([P, 2 * FOUT], F16)
                for k in range(2):
                    nc.sync.dma_start(w2t[:, k * FOUT:(k + 1) * FOUT],
                                      t_w2[k * P:(k + 1) * P, :])
                b1t = cs.tile([P, 2], F32)
                nc.sync.dma_start(b1t[:], t_b1[:])
            else:
                b2t = cs.tile([P, 1], F32)
                nc.sync.dma_start(b2t[:], t_b2[:])

            for b in range(NBLK):
                il = sb.tile([P, clo * 8], I16, tag="il")
                nc.sync.dma_start(il[:], t_il[b])
                ih = sb.tile([P, chi * 8], I16, tag="ih")
                nc.sync.dma_start(ih[:], t_ih[b])
                dst = sb.tile([P, cpbt], F16, tag="ds")
                nc.sync.dma_start(dst[:], t_ds[b])
                nrm = sb.tile([P, cpbt], F16, tag="nm")
                nc.sync.dma_start(nrm[:], t_nm[b])

                g = sb.tile([P, cpbt * elem], gd, tag="g")
                g3 = g[:].rearrange("p (c e) -> p c e", e=elem)
                # chunk 0: self loops, affine fetch from slot-ordered table
                nc.sync.dma_start(g[:, 0:elem], t_sf[b * P:(b + 1) * P, :])
                for (j0, nch) in lo_splits:
                    nc.gpsimd.dma_gather(
                        out_ap=g3[:, 1 + j0:1 + j0 + nch, :],
                        in_ap=tf_lo,
                        idxs_ap=il[:, j0 * 8:(j0 + nch) * 8],
                        num_idxs=nch * P,
                        num_idxs_reg=nch * P,
                        elem_size=elem,
                        queue_num=_next_q(),
                    )
                for (j0, nch) in hi_splits:
                    nc.gpsimd.dma_gather(
                        out_ap=g3[:, 1 + clo + j0:1 + clo + j0 + nch, :],
                        in_ap=tf_hi,
                        idxs_ap=ih[:, j0 * 8:(j0 + nch) * 8],
                        num_idxs=nch * P,
                        num_idxs_reg=nch * P,
                        elem_size=elem,
                        queue_num=_next_q(),
                    )

                seq = sb.tile([P, cpbt * P], F16, tag="seq")
                nc.vector.tensor_tensor(
                    out=seq[:].rearrange("p (c s) -> p c s", c=cpbt),
                    in0=dst[:].to_broadcast([P, cpbt, P]),
                    in1=colidx[:].rearrange("p (c s) -> p c s", c=cpbt),
                    op=mybir.AluOpType.is_equal)
                s = sb.tile([P, cpbt * P], F16, tag="s")
                nc.vector.tensor_tensor(
                    out=s[:].rearrange("p (c s) -> p c s", c=cpbt),
                    in0=seq[:].rearrange("p (c s) -> p c s", c=cpbt),
                    in1=nrm[:].to_broadcast([P, cpbt, P]),
                    op=mybir.AluOpType.mult)

                if is_b:
                    h0p = ps.tile([P, P], F32, tag="h0p")
                    h1p = ps.tile([P, P], F32, tag="h1p")
                    for j in range(cpbt):
                        nc.tensor.matmul(h0p[:], lhsT=g[:, j * elem:j * elem + P],
                                         rhs=s[:, j * P:(j + 1) * P],
                                         start=(j == 0), stop=(j == cpbt - 1))
                        nc.tensor.matmul(h1p[:], lhsT=g[:, j * elem + P:(j + 1) * elem],
                                         rhs=s[:, j * P:(j + 1) * P],
                                         start=(j == 0), stop=(j == cpbt - 1))
                    h0 = sb.tile([P, P], F16, tag="h0")
                    nc.scalar.activation(out=h0[:], in_=h0p[:],
                                         func=mybir.ActivationFunctionType.Relu,
                                         bias=b1t[:, 0:1], scale=1.0)
                    h1 = sb.tile([P, P], F16, tag="h1")
                    nc.scalar.activation(out=h1[:], in_=h1p[:],
                                         func=mybir.ActivationFunctionType.Relu,
                                         bias=b1t[:, 1:2], scale=1.0)
                    zp = ps.tile([P, P], F32, tag="zp")
                    nc.tensor.matmul(zp[:], lhsT=w2t[:, 0:FOUT], rhs=h0[:],
                                     start=True, stop=False)
                    nc.tensor.matmul(zp[:], lhsT=w2t[:, FOUT:2 * FOUT], rhs=h1[:],
                                     start=False, stop=True)
                    z = sb.tile([P, P], F16, tag="z")
                    nc.vector.tensor_copy(z[:], zp[:])
                    nc.sync.dma_start(t_o[:, b * P:(b + 1) * P], z[:])
                else:
                    op_ = ps.tile([P, P], F32, tag="op")
                    for j in range(cpbt):
                        nc.tensor.matmul(op_[:], lhsT=g[:, j * elem:(j + 1) * elem],
                                         rhs=s[:, j * P:(j + 1) * P],
                                         start=(j == 0), stop=(j == cpbt - 1))
                    ot = sb.tile([P, P], F32, tag="ot")
                    nc.scalar.activation(out=ot[:], in_=op_[:],
                                         func=mybir.ActivationFunctionType.Identity,
                                         bias=b2t[:, 0:1], scale=1.0)
                    nc.sync.dma_start(t_o[:, b * P:(b + 1) * P], ot[:])
    nc.compile()
    return nc


_KERNEL_CACHE = {}


def _get_kernels(clo, chi, lo_splits, hi_splits):
    key = (clo, chi)
    if key not in _KERNEL_CACHE:
        cfg = dict(clo=clo, chi=chi, lo_splits=lo_splits, hi_splits=hi_splits)
        _KERNEL_CACHE[key] = (
            _build_a(),
            _build_agg({**cfg, "is_b": True}),
            _build_agg({**cfg, "is_b": False}),
        )
    return _KERNEL_CACHE[key]


def kernel(x, edge_index, W1, b1, W2, b2):
    trace = bool(int(os.environ.get("GCN_TRACE", "0")))
    if trace:
        _ensure_ntff_hook()
    exec_ns = []

    def _run(nc, in_maps):
        res = run_bass_kernel_spmd(nc, in_maps, core_ids=list(range(NCORES)),
                                   trace=trace)
        if trace:
            exec_ns.append(res.exec_time_ns)
        return res.results

    x = np.asarray(x)
    edge_index = np.asarray(edge_index)
    W1 = np.asarray(W1, np.float32)
    b1 = np.asarray(b1, np.float32)
    W2 = np.asarray(W2, np.float32)
    b2 = np.asarray(b2, np.float32)

    pre, clo, chi, lo_splits, hi_splits = _preprocess(edge_index)
    cpbt = 1 + clo + chi
    nc_a, nc_b, nc_c = _get_kernels(clo, chi, lo_splits, hi_splits)

    # ---- launch A: T = x @ W1 (per-core node shard) ----
    w1_f16 = W1.astype(np.float16)
    in_a = []
    for c in range(NCORES):
        xs = np.zeros((NLOC_PAD, FIN), np.float16)
        xs[:NLOC] = x[c * NLOC:(c + 1) * NLOC].astype(np.float16)
        xtb = np.ascontiguousarray(
            xs.reshape(NBLK, P, _KC, P).transpose(0, 3, 2, 1).reshape(NBLK, P, FIN))
        in_a.append({"xtb": xtb, "w1": w1_f16})
    res_a = _run(nc_a, in_a)
    tfull = np.concatenate([res_a[c]["t_out"][:NLOC] for c in range(NCORES)], axis=0)
    tfull = np.ascontiguousarray(tfull)            # [N, HID] fp8e4m3

    colidx = np.tile(np.arange(P, dtype=np.float16)[None, :], (P, cpbt))
    colidx = np.ascontiguousarray(colidx.reshape(P, cpbt * P))

    # ---- launch B: h = relu(agg(T) + b1); Z^T = W2^T h^T ----
    w2_f16 = W2.astype(np.float16)
    b1c = np.stack([b1[:P], b1[P:]], axis=1).astype(np.float32)
    in_b = []
    for c in range(NCORES):
        tself = np.zeros((NLOC_PAD, HID), tfull.dtype)
        tself[pre[c]["perm"]] = tfull[c * NLOC:(c + 1) * NLOC]
        in_b.append({
            "tfull": tfull, "tself": tself,
            "idxlo": pre[c]["idxlo"], "idxhi": pre[c]["idxhi"],
            "dstslot": pre[c]["dstslot"], "normv": pre[c]["normv"],
            "colidx": colidx, "w2": w2_f16, "b1c": b1c,
        })
    res_b = _run(nc_b, in_b)
    zts = [res_b[c]["zt_out"] for c in range(NCORES)]
    zfull = np.concatenate(
        [zts[c].T[pre[c]["perm"]] for c in range(NCORES)], axis=0)
    zfull = np.ascontiguousarray(zfull)            # [N, FOUT] f16

    # ---- launch C: out = agg(Z) + b2 ----
    b2c = b2[:, None].astype(np.float32)
    in_c = [{
        "tfull": zfull, "tself": np.ascontiguousarray(zts[c].T),
        "idxlo": pre[c]["idxlo"], "idxhi": pre[c]["idxhi"],
        "dstslot": pre[c]["dstslot"], "normv": pre[c]["normv"],
        "colidx": colidx, "b2c": b2c,
    } for c in range(NCORES)]
    res_c = _run(nc_c, in_c)
    out = np.concatenate(
        [res_c[c]["ot_out"].T[pre[c]["perm"]] for c in range(NCORES)], axis=0)

    if trace:
        ns = [int(t) if t else 0 for t in exec_ns]
        print(f"GCN launch exec times (ns): {ns}  total: {sum(ns)}")
        kernel.last_exec_ns = ns
    return np.ascontiguousarray(out.astype(np.float32))

